# revision 40
# baseline (speedup 1.0000x reference)
"""Multi-head attention (B=2, S=2048, D=1024, H=16) on 8 Trainium2 NeuronCores.

Sharding: core i handles batch b = i//4 and head-group hg = i%4 (4 heads,
256 channels).  Per-head Q/K projection weights are replicated; the fc layer
is sharded over its contraction dim (each core contributes a partial y that
the host sums per batch).  Wv is folded into Wfc on the host (exact, since
softmax rows sum to 1 the bv contribution folds into bfc).

Device algorithm per core (all matmuls f32r, psum f32):
  - heads processed as PAIRS: head A on SBUF partitions 0-63, head B on
    64-127, so the K=64 score matmuls run as concurrent 64-row PE tiles
  - q'T/k'T projections: head B via a row-tiled M=128 matmul with
    block-diagonal weights, head A plain M=64 (scale 1/sqrt(64) folded in Wq)
  - scores computed transposed: S_t[k,q] = k'T_slice.T @ q'T (k on
    partitions); exp on ScalarE (no max-subtraction; |scores| <~ 2 so exp
    is safe), one N=1024 activation per k-tile covering both heads
  - AV: lhsT = [V_h | ones] (65 cols) so row 64 of the psum accumulator is
    the softmax denominator; accumulators are evacuated to SBUF and
    normalization is deferred one round (reciprocal -> ones-matmul
    partition-broadcast -> multiply); head B's normalized oT additionally
    goes through an identity-shift matmul to partitions 64-127
  - fc: y[s,c] accumulated over the two head-pairs with K=128 matmuls,
    emitted as single-matmul closures interleaved one-per-k-tile into the
    next q-window's attention so they hide under the ScalarE exp stream
"""


import sys

import numpy as np

if "/opt/trn_rl_repo" not in sys.path:
    sys.path.insert(0, "/opt/trn_rl_repo")

HEAD = 16
B, S, D = 2, 2048, 1024
HD = 64
HPC = 4          # heads per core
CH = HPC * HD    # channels per core
N_CORES = 8

_CACHE = {}
LAST_RESULTS = None


def _build():
    import concourse.tile as tile
    from concourse import bacc, mybir

    f32 = mybir.dt.float32
    f32r = mybir.dt.float32r
    EXP = mybir.ActivationFunctionType.Exp

    nc = bacc.Bacc("TRN2", target_bir_lowering=False, debug=False,
                   num_devices=N_CORES)

    # unused internal tensor whose name varies per retry: changes the BIR
    # content hash so a retry never reuses a possibly-corrupt cached NEFF
    nonce = _CACHE.get("nonce", 0)
    if nonce:
        nc.dram_tensor(f"retry_nonce_{nonce}", [1, 1], mybir.dt.float32)

    qt_d = nc.dram_tensor("qt", [CH, S], f32r, kind="ExternalInput")
    kt_d = nc.dram_tensor("kt", [CH, S], f32r, kind="ExternalInput")
    v1_d = nc.dram_tensor("v1", [S, 65 * HPC], f32r, kind="ExternalInput")
    wq_d = nc.dram_tensor("wqt", [2 * HD, 2 * HD], f32r, kind="ExternalInput")
    wk_d = nc.dram_tensor("wkt", [2 * HD, 2 * HD], f32r, kind="ExternalInput")
    bq_d = nc.dram_tensor("bq", [2 * HD, 1], f32, kind="ExternalInput")
    bk_d = nc.dram_tensor("bk", [2 * HD, 1], f32, kind="ExternalInput")
    wf_d = nc.dram_tensor("wfct", [CH, D], f32r, kind="ExternalInput")
    on_d = nc.dram_tensor("ones", [1, HD], f32r, kind="ExternalInput")
    ish_d = nc.dram_tensor("ishift", [HD, 2 * HD], f32r, kind="ExternalInput")
    y_d = nc.dram_tensor("y", [S, D], f32, kind="ExternalOutput")

    with tile.TileContext(nc) as tc, nc.allow_low_precision(
            reason="f32r tiles feed tensor-engine matmuls; psum stays f32"):
        with (
            tc.tile_pool(name="consts", bufs=1) as consts,
            tc.tile_pool(name="vpool", bufs=1) as vpool,
            tc.tile_pool(name="qk_in", bufs=1) as qk_in,
            tc.tile_pool(name="qk_proj", bufs=2) as qk_proj,
            tc.tile_pool(name="ot", bufs=1) as otp,
            tc.tile_pool(name="exp", bufs=3) as expp,
            tc.tile_pool(name="small", bufs=2) as small,
            tc.tile_pool(name="ysb", bufs=3) as ysb,
            tc.tile_pool(name="ps_score", bufs=2, space="PSUM") as ps_score,
            tc.tile_pool(name="ps_av", bufs=2, space="PSUM") as ps_av,
            tc.tile_pool(name="ps_misc", bufs=1, space="PSUM") as ps_misc,
        ):
            # ---------------- constants ----------------
            # only wk/wq gate the first projection; the rest can trail the
            # first input chunks
            wq_s = consts.tile([2 * HD, 2 * HD], f32r, tag="wq")
            wk_s = consts.tile([2 * HD, 2 * HD], f32r, tag="wk")
            bq_s = consts.tile([2 * HD, 1], f32, tag="bq")
            bk_s = consts.tile([2 * HD, 1], f32, tag="bk")
            ones_s = consts.tile([65, HD], f32r, tag="ones")
            ish_s = consts.tile([HD, 2 * HD], f32r, tag="ishift")

            def emit_late_consts():
                nc.sync.dma_start(out=bk_s, in_=bk_d[:, :])
                nc.sync.dma_start(out=bq_s, in_=bq_d[:, :])
                nc.sync.dma_start(out=ones_s[64:65, :], in_=on_d[:, :])
                nc.sync.dma_start(out=ish_s, in_=ish_d[:, :])
            # ---------------- projections ----------------
            # chunked input DMAs so the first proj matmul starts early;
            # j=0 inputs first, then v1 (needed from the first av), then the
            # remaining heads, then wfct (needed only by fc, much later)
            qp_s, kp_s = [], []
            v1_s = []
            wf_s = []

            def emit_proj(p):
                # head pair p: head 2p on partitions 0-63, head 2p+1 on
                # partitions 64-127 (concurrent 64x64 PE tiles T0 / T10)
                qt_t = qk_in.tile([2 * HD, S], f32r, tag="qt_in",
                                  name=f"qt_in{p}")
                kt_t = qk_in.tile([2 * HD, S], f32r, tag="kt_in",
                                  name=f"kt_in{p}")
                for c in range(S // 512):
                    sl = slice(512 * c, 512 * c + 512)
                    nc.sync.dma_start(out=kt_t[:, sl],
                                      in_=kt_d[128 * p:128 * p + 128, sl])
                    nc.sync.dma_start(out=qt_t[:, sl],
                                      in_=qt_d[128 * p:128 * p + 128, sl])
                    if p == 0 and c == 0:
                        nc.sync.dma_start(out=wk_s, in_=wk_d[:, :])
                        nc.sync.dma_start(out=wq_s, in_=wq_d[:, :])
                    if p == 0 and c == 1:
                        emit_late_consts()
                qp = qk_proj.tile([2 * HD, S], f32r, tag="qp", name=f"qp{p}")
                kp = qk_proj.tile([2 * HD, S], f32r, tag="kp", name=f"kp{p}")
                for qb in range(S // 512):
                    sl = slice(512 * qb, 512 * qb + 512)
                    # head B: row-tiled M=128 matmul with block-diag weights
                    # (only rows 64-127 valid); head A: plain M=64 matmul in
                    # a separate psum slot (same-bank double-write is a HW
                    # runtime error)
                    pk1 = ps_misc.tile([128, 512], f32, tag="misc",
                                       name=f"pk1{p}_{qb}")
                    nc.tensor.matmul(pk1, wk_s[64:128, :],
                                     kt_t[64:128, sl], start=True, stop=True)
                    nc.vector.tensor_scalar_add(kp[64:128, sl],
                                                pk1[64:128, :], bk_s[64:128])
                    pk2 = ps_misc.tile([128, 512], f32, tag="misc",
                                       name=f"pk2{p}_{qb}")
                    nc.tensor.matmul(pk2[0:64, :], wk_s[0:64, 0:64],
                                     kt_t[0:64, sl], start=True, stop=True)
                    nc.vector.tensor_scalar_add(kp[0:64, sl],
                                                pk2[0:64, :], bk_s[0:64])
                    pq1 = ps_misc.tile([128, 512], f32, tag="misc",
                                       name=f"pq1{p}_{qb}")
                    nc.tensor.matmul(pq1, wq_s[64:128, :],
                                     qt_t[64:128, sl], start=True, stop=True)
                    nc.vector.tensor_scalar_add(qp[64:128, sl],
                                                pq1[64:128, :], bq_s[64:128])
                    pq2 = ps_misc.tile([128, 512], f32, tag="misc",
                                       name=f"pq2{p}_{qb}")
                    nc.tensor.matmul(pq2[0:64, :], wq_s[0:64, 0:64],
                                     qt_t[0:64, sl], start=True, stop=True)
                    nc.vector.tensor_scalar_add(qp[0:64, sl],
                                                pq2[0:64, :], bq_s[0:64])
                qp_s.append(qp)
                kp_s.append(kp)

            emit_proj(0)
            for t in range(S // 128):
                v1t = vpool.tile([128, 65 * HPC], f32r, tag=f"v1_{t}",
                                 name=f"v1_{t}")
                nc.sync.dma_start(out=v1t, in_=v1_d[128 * t:128 * t + 128, :])
                v1_s.append(v1t)
            emit_proj(1)
            for pr in range(2):
                wfj = consts.tile([2 * HD, D], f32r, tag=f"wf{pr}",
                                  name=f"wf{pr}")
                nc.sync.dma_start(out=wfj,
                                  in_=wf_d[128 * pr:128 * pr + 128, :])
                wf_s.append(wfj)

            # ---------------- attention + interleaved fc ----------------
            oT_s = []
            for pr in range(2):
                oT = otp.tile([2 * HD, S], f32r, tag=f"oT{pr}", name=f"oT{pr}")
                oT_s.append(oT)

            NQB = S // 512           # outer q windows (512 wide)
            NKT = S // 128           # k tiles

            def emit_norm(p, qb, oc_t):
                # oc_t: sbuf [65, 1024] copy of the av accumulators for the
                # head pair (head 2p cols 0:512, head 2p+1 cols 512:1024;
                # row 64 = denominators). Normalize into the oT pair tile;
                # head B additionally goes through an identity-shift matmul
                # to land on partitions 64-127 (so fc can contract K=128).
                rsb = small.tile([65, 1024], f32r, tag="r",
                                 name=f"r{p}_{qb}")
                nc.vector.reciprocal(out=rsb[64:65, :], in_=oc_t[64:65, :])
                q0 = 512 * qb
                # head A (2p): normalize straight into rows 0-63
                rbpa = ps_misc.tile([HD, 512], f32, tag="rbp",
                                    name=f"rbpa{p}_{qb}")
                nc.tensor.matmul(rbpa, ones_s[64:65, :], rsb[64:65, 0:512],
                                 start=True, stop=True)
                nc.vector.tensor_mul(oT_s[p][0:64, q0:q0 + 512],
                                     rbpa, oc_t[0:64, 0:512])
                # head B (2p+1): normalize into a temp, shift to rows 64-127
                rbpb = ps_misc.tile([HD, 512], f32, tag="rbp",
                                    name=f"rbpb{p}_{qb}")
                nc.tensor.matmul(rbpb, ones_s[64:65, :], rsb[64:65, 512:1024],
                                 start=True, stop=True)
                oTb = small.tile([HD, 512], f32r, tag="oTb",
                                 name=f"oTb{p}_{qb}")
                nc.vector.tensor_mul(oTb, rbpb, oc_t[0:64, 512:1024])
                shp = ps_misc.tile([128, 512], f32, tag="rbp",
                                   name=f"shp{p}_{qb}")
                nc.tensor.matmul(shp, ish_s, oTb, start=True, stop=True)
                nc.vector.tensor_copy(oT_s[p][64:128, q0:q0 + 512],
                                      shp[64:128, :])

            # fc is emitted as single-matmul closures popped one per k-tile
            # iteration, so they never displace more than ~213ns of the
            # score->exp->av pipeline at a time.
            fc_state = {}

            def emit_fc_op(st, cb, pr, pool, tag):
                if cb == 0 and pr == 0:
                    fc_state["y_sb"] = ysb.tile([128, D], f32, tag="y",
                                                name=f"y{st}")
                if pr == 0:
                    fc_state["yp"] = pool.tile([128, 512], f32, tag=tag,
                                               name=f"yp{st}_{cb}")
                yp = fc_state["yp"]
                nc.tensor.matmul(
                    yp,
                    oT_s[pr][:, 128 * st:128 * st + 128],
                    wf_s[pr][:, 512 * cb:512 * cb + 512],
                    start=(pr == 0), stop=(pr == 1))
                if pr == 1:
                    y_sb = fc_state["y_sb"]
                    nc.vector.tensor_copy(y_sb[:, 512 * cb:512 * cb + 512], yp)
                    if cb == D // 512 - 1:
                        nc.sync.dma_start(
                            out=y_d[128 * st:128 * st + 128, :], in_=y_sb)

            # last q-window: pair-0 partials staged to SBUF during the final
            # attention round, pair-1 matmul + combine in the drain
            ya_st = {}

            def emit_fcA_op(st, cb):
                ypa = ps_misc.tile([128, 512], f32, tag="misc",
                                   name=f"ypa{st}_{cb}")
                nc.tensor.matmul(ypa,
                                 oT_s[0][:, 128 * st:128 * st + 128],
                                 wf_s[0][:, 512 * cb:512 * cb + 512],
                                 start=True, stop=True)
                ya = ysb.tile([128, 512], f32, tag="ya", bufs=8,
                              name=f"ya{st}_{cb}")
                nc.vector.tensor_copy(ya, ypa)
                ya_st[(st, cb)] = ya

            def emit_fcB_op(st, cb):
                if cb == 0:
                    fc_state["y_sb"] = ysb.tile([128, D], f32, tag="y",
                                                name=f"y{st}")
                ypb = ps_score.tile([128, 512], f32, tag="score",
                                    name=f"ypb{st}_{cb}")
                nc.tensor.matmul(ypb,
                                 oT_s[1][:, 128 * st:128 * st + 128],
                                 wf_s[1][:, 512 * cb:512 * cb + 512],
                                 start=True, stop=True)
                y_sb = fc_state["y_sb"]
                nc.vector.tensor_add(y_sb[:, 512 * cb:512 * cb + 512],
                                     ya_st[(st, cb)], ypb)
                if cb == D // 512 - 1:
                    nc.sync.dma_start(
                        out=y_d[128 * st:128 * st + 128, :], in_=y_sb)

            pending_norm = None
            fc_queue = []
            for qb in range(NQB):
                for p in range(2):
                    o_ps = []
                    for half in range(2):
                        o = ps_av.tile([65, 512], f32, tag="av",
                                       name=f"o{p}_{qb}_{half}")
                        o_ps.append(o)
                    q0 = 512 * qb
                    qa = qp_s[p][0:64, q0:q0 + 512]
                    qb_ = qp_s[p][64:128, q0:q0 + 512]
                    for kt in range(NKT):
                        ks = slice(128 * kt, 128 * kt + 128)
                        sc = ps_score.tile([128, 1024], f32, tag="score",
                                           name=f"sc{p}_{qb}_{kt}")
                        nc.tensor.matmul(sc[:, 0:512], kp_s[p][0:64, ks], qa,
                                         start=True, stop=True)
                        nc.tensor.matmul(sc[:, 512:1024],
                                         kp_s[p][64:128, ks], qb_,
                                         start=True, stop=True)
                        ex = expp.tile([128, 1024], f32r, tag="exp",
                                       name=f"ex{p}_{qb}_{kt}")
                        nc.scalar.activation(out=ex, in_=sc, func=EXP)
                        va = v1_s[kt][:, 65 * 2 * p:65 * 2 * p + 65]
                        vb = v1_s[kt][:, 65 * (2 * p + 1):65 * (2 * p + 1) + 65]
                        nc.tensor.matmul(o_ps[0], va, ex[:, 0:512],
                                         start=(kt == 0), stop=(kt == NKT - 1))
                        nc.tensor.matmul(o_ps[1], vb, ex[:, 512:1024],
                                         start=(kt == 0), stop=(kt == NKT - 1))
                        if kt == 2 and pending_norm is not None:
                            emit_norm(*pending_norm)
                            pending_norm = None
                            if qb == NQB - 1 and p == 1:
                                # pair-0 of the last window is normalized now;
                                # its fc partials can overlap this last round
                                for st_ in range(4 * qb, 4 * qb + 4):
                                    for cb_ in range(D // 512):
                                        fc_queue.append(
                                            lambda st=st_, cb=cb_:
                                                emit_fcA_op(st, cb))
                        if fc_queue:
                            fc_queue.pop(0)()
                    # evacuate the accumulators to SBUF quickly so the av
                    # psum slots free up; normalization is deferred
                    oc_t = small.tile([65, 1024], f32, tag="oc", bufs=4,
                                      name=f"oc{p}_{qb}")
                    nc.vector.tensor_copy(oc_t[:, 0:512], o_ps[0])
                    nc.vector.tensor_copy(oc_t[:, 512:1024], o_ps[1])
                    pending_norm = (p, qb, oc_t)
                # fc for this q-window needs both pairs' norms done
                emit_norm(*pending_norm)
                pending_norm = None
                # earlier windows' fc pops during later attention (misc psum
                # slot); the last window is split: pair-0 partials pop during
                # the final round, pair-1 + combine drain at the end
                if qb < NQB - 1:
                    for st in range(4 * qb, 4 * qb + 4):
                        for cb in range(D // 512):
                            for pr in range(2):
                                fc_queue.append(
                                    lambda st=st, cb=cb, pr=pr:
                                        emit_fc_op(st, cb, pr, ps_misc,
                                                   "misc"))
                # (last window's fcA ops are enqueued mid-round, above)
            while fc_queue:
                fc_queue.pop(0)()
            for st in range(S // 128 - 4, S // 128):
                for cb in range(D // 512):
                    emit_fcB_op(st, cb)

    nc.compile()
    return nc


def _prep(query, key, value, Wq, bq, Wk, bk, Wv, bv, Wfc, bfc):
    """Host-side sharding / layout prep. Returns (in_maps, bfc_eff)."""
    query = np.asarray(query, dtype=np.float32)
    key = np.asarray(key, dtype=np.float32)
    value = np.asarray(value, dtype=np.float32)
    Wq = np.asarray(Wq, np.float32); bq = np.asarray(bq, np.float32)
    Wk = np.asarray(Wk, np.float32); bk = np.asarray(bk, np.float32)
    Wv = np.asarray(Wv, np.float32); bv = np.asarray(bv, np.float32)
    Wfc = np.asarray(Wfc, np.float32); bfc = np.asarray(bfc, np.float32)

    scale = np.float32(1.0 / np.sqrt(HD))
    wq_t = np.ascontiguousarray(Wq.T) * scale        # [d, e], scale folded
    bq_sc = (bq * scale).reshape(HD, 1).astype(np.float32)
    wk_t = np.ascontiguousarray(Wk.T)
    bk_c = bk.reshape(HD, 1).astype(np.float32)
    # block-diagonal for head-pair packing: head A reads [0:64, 0:64],
    # head B reads rows 64:128 as [zeros | w] (row-tiled M=128 matmul)
    z = np.zeros((HD, HD), np.float32)
    wq_t2 = np.ascontiguousarray(np.block([[wq_t, z], [z, wq_t]]))
    wk_t2 = np.ascontiguousarray(np.block([[wk_t, z], [z, wk_t]]))
    bq_2 = np.ascontiguousarray(np.vstack([bq_sc, bq_sc]))
    bk_2 = np.ascontiguousarray(np.vstack([bk_c, bk_c]))

    # fold Wv / bv into fc
    A = np.empty((D, D), np.float32)
    bfc_eff = bfc.astype(np.float32).copy()
    for h in range(HEAD):
        Wfc_h = Wfc[:, HD * h:HD * h + HD]
        A[:, HD * h:HD * h + HD] = Wfc_h @ Wv
        bfc_eff += Wfc_h @ bv
    At = np.ascontiguousarray(A.T)                    # [ch, c]

    ishift = np.zeros((HD, 2 * HD), np.float32)
    ishift[np.arange(HD), HD + np.arange(HD)] = 1.0

    qT = np.ascontiguousarray(query.transpose(0, 2, 1))   # [B, D, S]
    kT = np.ascontiguousarray(key.transpose(0, 2, 1))

    in_maps = []
    for core in range(N_CORES):
        b, hg = core // 4, core % 4
        ch0 = CH * hg
        v1 = np.empty((S, 65 * HPC), np.float32)
        for j in range(HPC):
            v1[:, 65 * j:65 * j + 64] = value[b][:, ch0 + HD * j:ch0 + HD * j + HD]
            v1[:, 65 * j + 64] = 1.0
        in_maps.append({
            "qt": np.ascontiguousarray(qT[b][ch0:ch0 + CH]),
            "kt": np.ascontiguousarray(kT[b][ch0:ch0 + CH]),
            "v1": v1,
            "wqt": wq_t2,
            "wkt": wk_t2,
            "bq": bq_2,
            "bk": bk_2,
            "wfct": np.ascontiguousarray(At[ch0:ch0 + CH]),
            "ones": np.ones((1, HD), np.float32),
            "ishift": ishift,
        })
    return in_maps, bfc_eff


def _run_once(inputs):
    global LAST_RESULTS
    from concourse.bass_utils import run_bass_kernel_spmd

    if "nc" not in _CACHE:
        _CACHE["nc"] = _build()
    nc = _CACHE["nc"]

    in_maps, bfc_eff = _prep(**inputs)
    res = run_bass_kernel_spmd(nc, in_maps, core_ids=list(range(N_CORES)))
    LAST_RESULTS = res

    out = np.empty((B, S, D), np.float32)
    for b in range(B):
        acc = res.results[4 * b]["y"].astype(np.float32).copy()
        for hg in range(1, 4):
            acc += res.results[4 * b + hg]["y"]
        out[b] = acc + bfc_eff
    return out


def kernel(**inputs) -> np.ndarray:
    last_exc = None
    for attempt in range(3):
        try:
            out = _run_once(inputs)
            amax = float(np.abs(out).max())
            if np.isfinite(out).all() and 1e-6 < amax < 1e3:
                return out
            raise RuntimeError(f"implausible kernel output (absmax={amax})")
        except Exception as e:  # noqa: BLE001 - retry transient HW failures
            last_exc = e
            _CACHE.pop("nc", None)
            _CACHE["nonce"] = attempt + 1
    raise last_exc


# revision 48
# speedup vs baseline: 1.0134x; 1.0134x over previous
"""Multi-head attention (B=2, S=2048, D=1024, H=16) on 8 Trainium2 NeuronCores.

Sharding: core i handles batch b = i//4 and head-group hg = i%4 (4 heads,
256 channels).  Per-head Q/K projection weights are replicated; the fc layer
is sharded over its contraction dim (each core contributes a partial y that
the host sums per batch).  Wv is folded into Wfc on the host (exact, since
softmax rows sum to 1 the bv contribution folds into bfc).

Device algorithm per core (all matmuls f32r, psum f32):
  - heads processed as PAIRS: head A on SBUF partitions 0-63, head B on
    64-127, so the K=64 score matmuls run as concurrent 64-row PE tiles
  - q'T/k'T projections: head B via a row-tiled M=128 matmul with
    block-diagonal weights, head A plain M=64 (scale 1/sqrt(64) folded in Wq)
  - scores computed transposed: S_t[k,q] = k'T_slice.T @ q'T (k on
    partitions); exp on ScalarE (no max-subtraction; |scores| <~ 2 so exp
    is safe), one N=1024 activation per k-tile covering both heads
  - AV: lhsT = [V_h | ones] (65 cols) so row 64 of the psum accumulator is
    the softmax denominator; accumulators are evacuated to SBUF and
    normalization is deferred one round (reciprocal -> ones-matmul
    partition-broadcast -> multiply); head B's normalized oT additionally
    goes through an identity-shift matmul to partitions 64-127
  - fc: y[s,c] accumulated over the two head-pairs with K=128 matmuls,
    emitted as single-matmul closures interleaved one-per-k-tile into the
    next q-window's attention so they hide under the ScalarE exp stream
"""


import sys

import numpy as np

if "/opt/trn_rl_repo" not in sys.path:
    sys.path.insert(0, "/opt/trn_rl_repo")

HEAD = 16
B, S, D = 2, 2048, 1024
HD = 64
HPC = 4          # heads per core
CH = HPC * HD    # channels per core
N_CORES = 8

_CACHE = {}
LAST_RESULTS = None


def _build():
    import concourse.tile as tile
    from concourse import bacc, mybir

    f32 = mybir.dt.float32
    f32r = mybir.dt.float32r
    EXP = mybir.ActivationFunctionType.Exp

    nc = bacc.Bacc("TRN2", target_bir_lowering=False, debug=False,
                   num_devices=N_CORES)

    # unused internal tensor whose name varies per retry: changes the BIR
    # content hash so a retry never reuses a possibly-corrupt cached NEFF
    nonce = _CACHE.get("nonce", 0)
    if nonce:
        nc.dram_tensor(f"retry_nonce_{nonce}", [1, 1], mybir.dt.float32)

    qt_d = nc.dram_tensor("qt", [CH, S], f32r, kind="ExternalInput")
    kt_d = nc.dram_tensor("kt", [CH, S], f32r, kind="ExternalInput")
    v1_d = nc.dram_tensor("v1", [S, 65 * HPC], f32r, kind="ExternalInput")
    wq_d = nc.dram_tensor("wqt", [2 * HD, 2 * HD], f32r, kind="ExternalInput")
    wk_d = nc.dram_tensor("wkt", [2 * HD, 2 * HD], f32r, kind="ExternalInput")
    bq_d = nc.dram_tensor("bq", [2 * HD, 1], f32, kind="ExternalInput")
    bk_d = nc.dram_tensor("bk", [2 * HD, 1], f32, kind="ExternalInput")
    wf_d = nc.dram_tensor("wfct", [CH, D], f32r, kind="ExternalInput")
    on_d = nc.dram_tensor("ones", [1, HD], f32r, kind="ExternalInput")
    ish_d = nc.dram_tensor("ishift", [HD, 2 * HD], f32r, kind="ExternalInput")
    y_d = nc.dram_tensor("y", [S, D], f32, kind="ExternalOutput")

    with tile.TileContext(nc) as tc, nc.allow_low_precision(
            reason="f32r tiles feed tensor-engine matmuls; psum stays f32"):
        with (
            tc.tile_pool(name="consts", bufs=1) as consts,
            tc.tile_pool(name="vpool", bufs=1) as vpool,
            tc.tile_pool(name="qk_in", bufs=2) as qk_in,
            tc.tile_pool(name="qk_proj", bufs=2) as qk_proj,
            tc.tile_pool(name="ot", bufs=1) as otp,
            tc.tile_pool(name="exp", bufs=3) as expp,
            tc.tile_pool(name="small", bufs=2) as small,
            tc.tile_pool(name="ysb", bufs=3) as ysb,
            tc.tile_pool(name="ps_score", bufs=2, space="PSUM") as ps_score,
            tc.tile_pool(name="ps_av", bufs=2, space="PSUM") as ps_av,
            tc.tile_pool(name="ps_misc", bufs=1, space="PSUM") as ps_misc,
        ):
            # ---------------- constants ----------------
            # only wk/wq gate the first projection; the rest can trail the
            # first input chunks
            wq_s = consts.tile([2 * HD, 2 * HD], f32r, tag="wq")
            wk_s = consts.tile([2 * HD, 2 * HD], f32r, tag="wk")
            bq_s = consts.tile([2 * HD, 1], f32, tag="bq")
            bk_s = consts.tile([2 * HD, 1], f32, tag="bk")
            ones_s = consts.tile([65, HD], f32r, tag="ones")
            ish_s = consts.tile([HD, 2 * HD], f32r, tag="ishift")

            def emit_late_consts():
                nc.sync.dma_start(out=bk_s, in_=bk_d[:, :])
                nc.sync.dma_start(out=bq_s, in_=bq_d[:, :])
                nc.sync.dma_start(out=ones_s[64:65, :], in_=on_d[:, :])
                nc.sync.dma_start(out=ish_s, in_=ish_d[:, :])
            # ---------------- projections ----------------
            # chunked input DMAs so the first proj matmul starts early;
            # j=0 inputs first, then v1 (needed from the first av), then the
            # remaining heads, then wfct (needed only by fc, much later)
            qp_s, kp_s = [], []
            v1_s = []
            wf_s = []
            deferred_qt = []

            def emit_proj(p):
                # head pair p: head 2p on partitions 0-63, head 2p+1 on
                # partitions 64-127 (concurrent 64x64 PE tiles T0 / T10)
                qt_t = qk_in.tile([2 * HD, S], f32r, tag="qt_in",
                                  name=f"qt_in{p}")
                kt_t = qk_in.tile([2 * HD, S], f32r, tag="kt_in",
                                  name=f"kt_in{p}")
                if p == 0:
                    # critical-path order: the first k-proj matmul needs only
                    # wk + kt chunk 0; scores consume kp chunk-by-chunk, but
                    # qt chunks 1-3 are not needed until the second q-window,
                    # so defer them until after the v1 loads
                    nc.sync.dma_start(out=wk_s, in_=wk_d[:, :])
                    nc.sync.dma_start(out=kt_t[:, 0:512],
                                      in_=kt_d[0:128, 0:512])
                    nc.sync.dma_start(out=wq_s, in_=wq_d[:, :])
                    nc.sync.dma_start(out=qt_t[:, 0:512],
                                      in_=qt_d[0:128, 0:512])
                    emit_late_consts()
                    for c in range(1, S // 512):
                        sl = slice(512 * c, 512 * c + 512)
                        nc.sync.dma_start(out=kt_t[:, sl],
                                          in_=kt_d[0:128, sl])
                    for c in range(1, S // 512):
                        sl = slice(512 * c, 512 * c + 512)
                        nc.sync.dma_start(out=qt_t[:, sl],
                                          in_=qt_d[0:128, sl])
                else:
                    for c in range(S // 512):
                        sl = slice(512 * c, 512 * c + 512)
                        nc.sync.dma_start(out=kt_t[:, sl],
                                          in_=kt_d[128 * p:128 * p + 128, sl])
                        nc.sync.dma_start(out=qt_t[:, sl],
                                          in_=qt_d[128 * p:128 * p + 128, sl])
                qp = qk_proj.tile([2 * HD, S], f32r, tag="qp", name=f"qp{p}")
                kp = qk_proj.tile([2 * HD, S], f32r, tag="kp", name=f"kp{p}")
                for qb in range(S // 512):
                    sl = slice(512 * qb, 512 * qb + 512)
                    # head B: row-tiled M=128 matmul with block-diag weights
                    # (only rows 64-127 valid); head A: plain M=64 matmul in
                    # a separate psum slot (same-bank double-write is a HW
                    # runtime error)
                    pk1 = ps_misc.tile([128, 512], f32, tag="misc",
                                       name=f"pk1{p}_{qb}")
                    nc.tensor.matmul(pk1, wk_s[64:128, :],
                                     kt_t[64:128, sl], start=True, stop=True)
                    nc.vector.tensor_scalar_add(kp[64:128, sl],
                                                pk1[64:128, :], bk_s[64:128])
                    pk2 = ps_misc.tile([128, 512], f32, tag="misc",
                                       name=f"pk2{p}_{qb}")
                    nc.tensor.matmul(pk2[0:64, :], wk_s[0:64, 0:64],
                                     kt_t[0:64, sl], start=True, stop=True)
                    nc.vector.tensor_scalar_add(kp[0:64, sl],
                                                pk2[0:64, :], bk_s[0:64])
                    pq1 = ps_misc.tile([128, 512], f32, tag="misc",
                                       name=f"pq1{p}_{qb}")
                    nc.tensor.matmul(pq1, wq_s[64:128, :],
                                     qt_t[64:128, sl], start=True, stop=True)
                    nc.vector.tensor_scalar_add(qp[64:128, sl],
                                                pq1[64:128, :], bq_s[64:128])
                    pq2 = ps_misc.tile([128, 512], f32, tag="misc",
                                       name=f"pq2{p}_{qb}")
                    nc.tensor.matmul(pq2[0:64, :], wq_s[0:64, 0:64],
                                     qt_t[0:64, sl], start=True, stop=True)
                    nc.vector.tensor_scalar_add(qp[0:64, sl],
                                                pq2[0:64, :], bq_s[0:64])
                qp_s.append(qp)
                kp_s.append(kp)

            emit_proj(0)
            for t in range(S // 128):
                v1t = vpool.tile([128, 65 * HPC], f32r, tag=f"v1_{t}",
                                 name=f"v1_{t}")
                nc.sync.dma_start(out=v1t, in_=v1_d[128 * t:128 * t + 128, :])
                v1_s.append(v1t)
            for fn in deferred_qt:
                fn()
            emit_proj(1)
            for pr in range(2):
                wfj = consts.tile([2 * HD, D], f32r, tag=f"wf{pr}",
                                  name=f"wf{pr}")
                nc.sync.dma_start(out=wfj,
                                  in_=wf_d[128 * pr:128 * pr + 128, :])
                wf_s.append(wfj)

            # ---------------- attention + interleaved fc ----------------
            oT_s = []
            for pr in range(2):
                oT = otp.tile([2 * HD, S], f32r, tag=f"oT{pr}", name=f"oT{pr}")
                oT_s.append(oT)

            NQB = S // 512           # outer q windows (512 wide)
            NKT = S // 128           # k tiles

            def emit_norm(p, qb, oc_t):
                # oc_t: sbuf [65, 1024] copy of the av accumulators for the
                # head pair (head 2p cols 0:512, head 2p+1 cols 512:1024;
                # row 64 = denominators). Normalize into the oT pair tile;
                # head B additionally goes through an identity-shift matmul
                # to land on partitions 64-127 (so fc can contract K=128).
                rsb = small.tile([65, 1024], f32r, tag="r",
                                 name=f"r{p}_{qb}")
                nc.vector.reciprocal(out=rsb[64:65, :], in_=oc_t[64:65, :])
                q0 = 512 * qb
                # head A (2p): normalize straight into rows 0-63
                rbpa = ps_misc.tile([HD, 512], f32, tag="rbp",
                                    name=f"rbpa{p}_{qb}")
                nc.tensor.matmul(rbpa, ones_s[64:65, :], rsb[64:65, 0:512],
                                 start=True, stop=True)
                nc.vector.tensor_mul(oT_s[p][0:64, q0:q0 + 512],
                                     rbpa, oc_t[0:64, 0:512])
                # head B (2p+1): normalize into a temp, shift to rows 64-127
                rbpb = ps_misc.tile([HD, 512], f32, tag="rbp",
                                    name=f"rbpb{p}_{qb}")
                nc.tensor.matmul(rbpb, ones_s[64:65, :], rsb[64:65, 512:1024],
                                 start=True, stop=True)
                oTb = small.tile([HD, 512], f32r, tag="oTb",
                                 name=f"oTb{p}_{qb}")
                nc.vector.tensor_mul(oTb, rbpb, oc_t[0:64, 512:1024])
                shp = ps_misc.tile([128, 512], f32, tag="rbp",
                                   name=f"shp{p}_{qb}")
                nc.tensor.matmul(shp, ish_s, oTb, start=True, stop=True)
                nc.vector.tensor_copy(oT_s[p][64:128, q0:q0 + 512],
                                      shp[64:128, :])

            # fc is emitted as single-matmul closures popped one per k-tile
            # iteration, so they never displace more than ~213ns of the
            # score->exp->av pipeline at a time.
            fc_state = {}

            def emit_fc_op(st, cb, pr, pool, tag):
                if cb == 0 and pr == 0:
                    fc_state["y_sb"] = ysb.tile([128, D], f32, tag="y",
                                                name=f"y{st}")
                if pr == 0:
                    fc_state["yp"] = pool.tile([128, 512], f32, tag=tag,
                                               name=f"yp{st}_{cb}")
                yp = fc_state["yp"]
                nc.tensor.matmul(
                    yp,
                    oT_s[pr][:, 128 * st:128 * st + 128],
                    wf_s[pr][:, 512 * cb:512 * cb + 512],
                    start=(pr == 0), stop=(pr == 1))
                if pr == 1:
                    y_sb = fc_state["y_sb"]
                    nc.vector.tensor_copy(y_sb[:, 512 * cb:512 * cb + 512], yp)
                    if cb == D // 512 - 1:
                        nc.sync.dma_start(
                            out=y_d[128 * st:128 * st + 128, :], in_=y_sb)

            # last q-window: pair-0 partials staged to SBUF during the final
            # attention round, pair-1 matmul + combine in the drain
            ya_st = {}

            def emit_fcA_op(st, cb):
                ypa = ps_misc.tile([128, 512], f32, tag="misc",
                                   name=f"ypa{st}_{cb}")
                nc.tensor.matmul(ypa,
                                 oT_s[0][:, 128 * st:128 * st + 128],
                                 wf_s[0][:, 512 * cb:512 * cb + 512],
                                 start=True, stop=True)
                ya = ysb.tile([128, 512], f32, tag="ya", bufs=8,
                              name=f"ya{st}_{cb}")
                nc.vector.tensor_copy(ya, ypa)
                ya_st[(st, cb)] = ya

            def emit_fcB_op(st, cb):
                if cb == 0:
                    fc_state["y_sb"] = ysb.tile([128, D], f32, tag="y",
                                                name=f"y{st}")
                # rotate three psum slots (2x score + the idle misc bank) so
                # the drain-phase matmul/copy chains pipeline deeper
                if (2 * st + cb) % 3 == 2:
                    ypb = ps_misc.tile([128, 512], f32, tag="misc",
                                       name=f"ypb{st}_{cb}")
                else:
                    ypb = ps_score.tile([128, 512], f32, tag="score",
                                        name=f"ypb{st}_{cb}")
                nc.tensor.matmul(ypb,
                                 oT_s[1][:, 128 * st:128 * st + 128],
                                 wf_s[1][:, 512 * cb:512 * cb + 512],
                                 start=True, stop=True)
                y_sb = fc_state["y_sb"]
                nc.vector.tensor_add(y_sb[:, 512 * cb:512 * cb + 512],
                                     ya_st[(st, cb)], ypb)
                if cb == D // 512 - 1:
                    nc.sync.dma_start(
                        out=y_d[128 * st:128 * st + 128, :], in_=y_sb)

            pending_norm = None
            fc_queue = []
            for qb in range(NQB):
                for p in range(2):
                    o_ps = []
                    for half in range(2):
                        o = ps_av.tile([65, 512], f32, tag="av",
                                       name=f"o{p}_{qb}_{half}")
                        o_ps.append(o)
                    q0 = 512 * qb
                    qa = qp_s[p][0:64, q0:q0 + 512]
                    qb_ = qp_s[p][64:128, q0:q0 + 512]
                    for kt in range(NKT):
                        ks = slice(128 * kt, 128 * kt + 128)
                        sc = ps_score.tile([128, 1024], f32, tag="score",
                                           name=f"sc{p}_{qb}_{kt}")
                        nc.tensor.matmul(sc[:, 0:512], kp_s[p][0:64, ks], qa,
                                         start=True, stop=True)
                        nc.tensor.matmul(sc[:, 512:1024],
                                         kp_s[p][64:128, ks], qb_,
                                         start=True, stop=True)
                        ex = expp.tile([128, 1024], f32r, tag="exp",
                                       name=f"ex{p}_{qb}_{kt}")
                        nc.scalar.activation(out=ex, in_=sc, func=EXP)
                        va = v1_s[kt][:, 65 * 2 * p:65 * 2 * p + 65]
                        vb = v1_s[kt][:, 65 * (2 * p + 1):65 * (2 * p + 1) + 65]
                        nc.tensor.matmul(o_ps[0], va, ex[:, 0:512],
                                         start=(kt == 0), stop=(kt == NKT - 1))
                        nc.tensor.matmul(o_ps[1], vb, ex[:, 512:1024],
                                         start=(kt == 0), stop=(kt == NKT - 1))
                        if kt == 2 and pending_norm is not None:
                            emit_norm(*pending_norm)
                            pending_norm = None
                            if qb == NQB - 1 and p == 1:
                                # pair-0 of the last window is normalized now;
                                # its fc partials can overlap this last round
                                for st_ in range(4 * qb, 4 * qb + 4):
                                    for cb_ in range(D // 512):
                                        fc_queue.append(
                                            lambda st=st_, cb=cb_:
                                                emit_fcA_op(st, cb))
                        if fc_queue:
                            fc_queue.pop(0)()
                    # evacuate the accumulators to SBUF quickly so the av
                    # psum slots free up; normalization is deferred
                    oc_t = small.tile([65, 1024], f32, tag="oc", bufs=4,
                                      name=f"oc{p}_{qb}")
                    nc.vector.tensor_copy(oc_t[:, 0:512], o_ps[0])
                    nc.vector.tensor_copy(oc_t[:, 512:1024], o_ps[1])
                    pending_norm = (p, qb, oc_t)
                # fc for this q-window needs both pairs' norms done
                emit_norm(*pending_norm)
                pending_norm = None
                # earlier windows' fc pops during later attention (misc psum
                # slot); the last window is split: pair-0 partials pop during
                # the final round, pair-1 + combine drain at the end
                if qb < NQB - 1:
                    for st in range(4 * qb, 4 * qb + 4):
                        for cb in range(D // 512):
                            for pr in range(2):
                                fc_queue.append(
                                    lambda st=st, cb=cb, pr=pr:
                                        emit_fc_op(st, cb, pr, ps_misc,
                                                   "misc"))
                # (last window's fcA ops are enqueued mid-round, above)
            while fc_queue:
                fc_queue.pop(0)()
            for st in range(S // 128 - 4, S // 128):
                for cb in range(D // 512):
                    emit_fcB_op(st, cb)

    nc.compile()
    return nc


def _prep(query, key, value, Wq, bq, Wk, bk, Wv, bv, Wfc, bfc):
    """Host-side sharding / layout prep. Returns (in_maps, bfc_eff)."""
    query = np.asarray(query, dtype=np.float32)
    key = np.asarray(key, dtype=np.float32)
    value = np.asarray(value, dtype=np.float32)
    Wq = np.asarray(Wq, np.float32); bq = np.asarray(bq, np.float32)
    Wk = np.asarray(Wk, np.float32); bk = np.asarray(bk, np.float32)
    Wv = np.asarray(Wv, np.float32); bv = np.asarray(bv, np.float32)
    Wfc = np.asarray(Wfc, np.float32); bfc = np.asarray(bfc, np.float32)

    scale = np.float32(1.0 / np.sqrt(HD))
    wq_t = np.ascontiguousarray(Wq.T) * scale        # [d, e], scale folded
    bq_sc = (bq * scale).reshape(HD, 1).astype(np.float32)
    wk_t = np.ascontiguousarray(Wk.T)
    bk_c = bk.reshape(HD, 1).astype(np.float32)
    # block-diagonal for head-pair packing: head A reads [0:64, 0:64],
    # head B reads rows 64:128 as [zeros | w] (row-tiled M=128 matmul)
    z = np.zeros((HD, HD), np.float32)
    wq_t2 = np.ascontiguousarray(np.block([[wq_t, z], [z, wq_t]]))
    wk_t2 = np.ascontiguousarray(np.block([[wk_t, z], [z, wk_t]]))
    bq_2 = np.ascontiguousarray(np.vstack([bq_sc, bq_sc]))
    bk_2 = np.ascontiguousarray(np.vstack([bk_c, bk_c]))

    # fold Wv / bv into fc
    A = np.empty((D, D), np.float32)
    bfc_eff = bfc.astype(np.float32).copy()
    for h in range(HEAD):
        Wfc_h = Wfc[:, HD * h:HD * h + HD]
        A[:, HD * h:HD * h + HD] = Wfc_h @ Wv
        bfc_eff += Wfc_h @ bv
    At = np.ascontiguousarray(A.T)                    # [ch, c]

    ishift = np.zeros((HD, 2 * HD), np.float32)
    ishift[np.arange(HD), HD + np.arange(HD)] = 1.0

    qT = np.ascontiguousarray(query.transpose(0, 2, 1))   # [B, D, S]
    kT = np.ascontiguousarray(key.transpose(0, 2, 1))

    in_maps = []
    for core in range(N_CORES):
        b, hg = core // 4, core % 4
        ch0 = CH * hg
        v1 = np.empty((S, 65 * HPC), np.float32)
        for j in range(HPC):
            v1[:, 65 * j:65 * j + 64] = value[b][:, ch0 + HD * j:ch0 + HD * j + HD]
            v1[:, 65 * j + 64] = 1.0
        in_maps.append({
            "qt": np.ascontiguousarray(qT[b][ch0:ch0 + CH]),
            "kt": np.ascontiguousarray(kT[b][ch0:ch0 + CH]),
            "v1": v1,
            "wqt": wq_t2,
            "wkt": wk_t2,
            "bq": bq_2,
            "bk": bk_2,
            "wfct": np.ascontiguousarray(At[ch0:ch0 + CH]),
            "ones": np.ones((1, HD), np.float32),
            "ishift": ishift,
        })
    return in_maps, bfc_eff


def _run_once(inputs):
    global LAST_RESULTS
    from concourse.bass_utils import run_bass_kernel_spmd

    if "nc" not in _CACHE:
        _CACHE["nc"] = _build()
    nc = _CACHE["nc"]

    in_maps, bfc_eff = _prep(**inputs)
    res = run_bass_kernel_spmd(nc, in_maps, core_ids=list(range(N_CORES)))
    LAST_RESULTS = res

    out = np.empty((B, S, D), np.float32)
    for b in range(B):
        acc = res.results[4 * b]["y"].astype(np.float32).copy()
        for hg in range(1, 4):
            acc += res.results[4 * b + hg]["y"]
        out[b] = acc + bfc_eff
    return out


def kernel(**inputs) -> np.ndarray:
    last_exc = None
    for attempt in range(3):
        try:
            out = _run_once(inputs)
            amax = float(np.abs(out).max())
            if np.isfinite(out).all() and 1e-6 < amax < 1e3:
                return out
            raise RuntimeError(f"implausible kernel output (absmax={amax})")
        except Exception as e:  # noqa: BLE001 - retry transient HW failures
            last_exc = e
            _CACHE.pop("nc", None)
            _CACHE["nonce"] = attempt + 1
    raise last_exc
